# revision 8
# baseline (speedup 1.0000x reference)
"""Additive-attention kernel for Trainium2 (8 NeuronCores, SPMD).

Problem (per batch b of B=4):
    xt      = x[b].T                                  # (N=512, D=96)
    g1      = xt @ Wg1.T                              # (512, 256)
    g2      = xt @ Wg2.T                              # (512, 256)
    score   = sum_a Wa[a] * tanh(g1[n,a] + g2[m,a] + bg[a])    # (512, 512)
    att     = sigmoid(score + Wa_b + ba)
    out[b]  = att @ xt                                # (512, 96)

Sharding: core c handles batch b = c//2 and query-rows n in
[(c%2)*256, (c%2)*256+256).  Keys are PERMUTED per core (own-half keys
first) so the query slice xq is a fixed-offset view of the key tile;
the sum over keys is permutation-invariant.

Algorithm (v7): tanh(u+v) ~= sum_{j in 1,3,5} BJ_j * sin(j*S*(u+v)),
S = pi/8.4 (weighted LSQ fit of tanh on |t|<=9, w=N(0,1.4^2)+0.05).
Each harmonic separates, so the N x N score becomes matmuls over a
contraction of (a, j, sin|cos) = 1536.  ACT's sin spline only covers
|x| < 4 (profile exponent buckets), so ONLY the j=1 seeds may use ACT
Sin; j=3,5 come from the f16 DVE recurrence
    f3 = f1*(2cos2t +- 1),  f5 = f3*2cos2t - f1.

Scheduling notes (hard-won):
- Engines only start after the preamble barrier (~7.0us); a HWDGE
  dma_start costs ~0.6-0.7us of descriptor-gen ON the issuing engine
  and the first bytes land ~0.6us later.  vin is ONE transfer on the
  sync ring; xkTP(+f16 bias columns) goes on the scalar HWDGE ring
  right after the silu dummy, so it never blocks vin.
- A single silu-dummy activation forces ONE load of silu_and_others
  (the only ACT table set with sin AND tanh).
- Tile tracks dependencies at TILE granularity, so sin/cos seed lanes
  live in separate tiles (false WAR between ACT lane writes and DVE
  readers otherwise serializes the seed chain).
- PE warms up (HAM 4/8 -> 8/8 needs ~3.4us of sustained activity) on
  ones-matmuls into the thv PSUM region before the real theta MMs.
- sigmoid rewritten 0.5+0.5*tanh(0.5*s): the 0.5 offset becomes a
  ones-colsum matmul and both 0.5 factors are pre-folded into xkTP on
  the host, so out = fos PSUM directly (ACT copies PSUM->SBUF f16).
"""

import numpy as np

B, D, N, A = 4, 96, 512, 256
NH = N // 2          # query rows per core
NCORES = 8

JS = (1, 3, 5)
FL = 8.4
FS = float(np.pi / FL)
BJ = {1: 1.206938, 3: 0.263773, 5: 0.089706}

_cache = {}


def _build_nc_v7(bg_zero=False):
    import concourse.bacc as bacc
    import concourse.mybir as mybir
    from concourse import tile

    f32 = mybir.dt.float32
    f16 = mybir.dt.float16
    AF = mybir.ActivationFunctionType
    MULT = mybir.AluOpType.mult
    ADD = mybir.AluOpType.add

    nc = bacc.Bacc("TRN2", target_bir_lowering=False)

    # xkb = xkTP [128, 4*D] ++ f16 bias columns:
    #   wav(2) sgb(1) [+ bg: b1s(2) b1c(2)]
    NBC = 3 if bg_zero else 7
    vin = nc.dram_tensor("vin", [D, 2 * A + N], f16, kind="ExternalInput")
    xkb = nc.dram_tensor("xkb", [128, 4 * D + NBC], f16,
                         kind="ExternalInput")
    out = nc.dram_tensor("out", [NH, D], f16, kind="ExternalOutput")

    FU = NH * 2          # 512: u-side feature width (2 a-chunks)

    with tile.TileContext(nc) as tc:
        with (
            tc.tile_pool(name="consts", bufs=1) as consts,
            tc.tile_pool(name="feat", bufs=1) as feat,
            tc.tile_pool(name="gps", bufs=1, space="PSUM") as gps,
            tc.tile_pool(name="scps", bufs=1, space="PSUM") as scps,
            tc.tile_pool(name="opool", bufs=1) as opool,
        ):
            vin_sb = consts.tile([D, 2 * A + N], f16, tag="vin")
            xkb_sb = consts.tile([128, 4 * D + NBC], f16, tag="xkb")
            # vin layout: [w1 | xk | w2]; xq = first NH key columns
            w1_sb = vin_sb[:, :A]
            xk_sb = vin_sb[:, A:A + N]
            xq_sb = xk_sb[:, :NH]
            w2_sb = vin_sb[:, A + N:]
            xkT_sb = xkb_sb[:, :4 * D].rearrange("p (mb d) -> p mb d", d=D)
            # bias columns ride as f16; widen to f32 once on-device
            # (tensor_scalar scalar1 and ACT bias APs must be f32)
            biasf = consts.tile([128, NBC], f32, tag="biasf")
            wav_sb = biasf[:, 0:2]
            sgb_sb = biasf[:, 2:3]
            if not bg_zero:
                b1s_sb = biasf[:, 3:5]
                b1c_sb = biasf[:, 5:7]

            # gpsimd owns the const memsets (it is otherwise idle and
            # starts right after the barrier, freeing DVE's queue)
            ones = consts.tile([128, 512], f16, tag="ones")
            nc.gpsimd.memset(ones[:], 1.0)
            hpi = consts.tile([128, 1], f32, tag="hpi")
            nc.gpsimd.memset(hpi[:], float(np.pi / 2))
            dsil = consts.tile([128, 1], f32, tag="dsil")
            nc.gpsimd.memset(dsil[:], 0.0)

            # silu-dummy: forces ONE load of silu_and_others, then the
            # xkb DMA rides the scalar HWDGE ring (vin owns sync's)
            nc.scalar.activation(dsil[:], dsil[:], AF.Silu)
            nc.sync.dma_start(vin_sb[:], vin.ap())
            nc.scalar.dma_start(xkb_sb[:], xkb.ap())

            thu = gps.tile([128, FU], f32, tag="thu", name="thu")
            thv = [gps.tile([128, N], f32, tag=f"thv{c}", name=f"thv{c}")
                   for c in range(2)]
            # PE warmup into thv0's region (overwritten by the real MM)
            for _ in range(3):
                nc.tensor.matmul(thv[0][:], ones[:, :128], ones[:])
            for c in range(2):
                nc.tensor.matmul(thu[:, c * NH:(c + 1) * NH],
                                 w1_sb[:, c * 128:(c + 1) * 128], xq_sb[:])
            for c in range(2):
                nc.tensor.matmul(thv[c][:],
                                 w2_sb[:, c * 128:(c + 1) * 128], xk_sb[:])

            # constant half-sum term: fos[nb] starts as sum_m xkTP[m, :]
            fos = [gps.tile([128, D], f32, tag="fo", name=f"fo{nb}")
                   for nb in range(2)]
            for mb in range(4):
                for nb in range(2):
                    nc.tensor.matmul(
                        fos[nb][:], ones[:, :128], xkT_sb[:, mb, :],
                        start=(mb == 0), stop=False, skip_group_check=True,
                    )

            # per-lane feature tiles (separate tiles per sin|cos lane so
            # ACT lane writes never false-serialize against DVE readers)
            # u side: [128, FU]; v side: [128, 2(chunk), N]
            cus = {j: feat.tile([128, FU], f16, tag=f"cus{j}", name=f"cus{j}") for j in JS}
            cuc = {j: feat.tile([128, FU], f16, tag=f"cuc{j}", name=f"cuc{j}") for j in JS}
            cvs = {j: feat.tile([128, 2, N], f16, tag=f"cvs{j}", name=f"cvs{j}") for j in JS}
            cvc = {j: feat.tile([128, 2, N], f16, tag=f"cvc{j}", name=f"cvc{j}") for j in JS}
            uss = {j: feat.tile([128, FU], f16, tag=f"uss{j}", name=f"uss{j}") for j in JS}
            usc = {j: feat.tile([128, FU], f16, tag=f"usc{j}", name=f"usc{j}") for j in JS}

            def useed(lane, tile_):
                if bg_zero:
                    bias = hpi[:] if lane else 0.0
                    nc.scalar.activation(tile_[:], thu[:], AF.Sin, bias=bias)
                else:
                    bl = b1c_sb if lane else b1s_sb
                    for c in range(2):
                        nc.scalar.activation(
                            tile_[:, c * NH:(c + 1) * NH],
                            thu[:, c * NH:(c + 1) * NH], AF.Sin,
                            bias=bl[:, c:c + 1])

            def vseed(lane, tile_, c):
                if bg_zero:
                    bias = hpi[:] if lane else 0.0
                else:
                    bias = (b1c_sb if lane else b1s_sb)[:, c:c + 1]
                nc.scalar.activation(tile_[:, c, :], thv[c][:], AF.Sin,
                                     bias=bias)

            useed(1, cuc[1])
            useed(0, cus[1])
            vseed(1, cvc[1], 0)
            vseed(1, cvc[1], 1)
            vseed(0, cvs[1], 0)
            vseed(0, cvs[1], 1)

            def uscale(j):
                for c in range(2):
                    for st, ct in ((uss, cus), (usc, cuc)):
                        nc.vector.tensor_scalar(
                            st[j][:, c * NH:(c + 1) * NH],
                            ct[j][:, c * NH:(c + 1) * NH],
                            wav_sb[:, c:c + 1], float(BJ[j]), MULT, MULT)

            sc = [scps.tile([128, NH], f32, tag=f"sc{mb}", name=f"sc{mb}")
                  for mb in range(4)]

            def score_mms(j, first=False):
                lhs = {0: cvc[j], 1: cvs[j]}  # fn=0 pairs us-sin x cv-cos
                rhs = {0: uss[j], 1: usc[j]}
                for fn in range(2):
                    for c in range(2):
                        for mb in range(4):
                            nc.tensor.matmul(
                                sc[mb][:],
                                lhs[fn][:, c, mb * 128:(mb + 1) * 128],
                                rhs[fn][:, c * NH:(c + 1) * NH],
                                start=(first and fn == 0 and c == 0),
                                stop=False,
                                skip_group_check=True,
                            )

            # ---- DVE program (emission order = engine order) ----
            squ = feat.tile([128, FU], f16, tag="squ")
            m3up = feat.tile([128, FU], f16, tag="m3up")
            m3um = feat.tile([128, FU], f16, tag="m3um")
            t2u = feat.tile([128, FU], f16, tag="t2u")
            nc.vector.tensor_copy(biasf[:], xkb_sb[:, 4 * D:])
            nc.vector.tensor_mul(squ[:], cuc[1][:], cuc[1][:])
            nc.vector.tensor_scalar(m3up[:], squ[:], 4.0, -1.0, MULT, ADD)
            nc.vector.tensor_scalar(m3um[:], squ[:], 4.0, -3.0, MULT, ADD)
            nc.vector.tensor_scalar(t2u[:], squ[:], 4.0, -2.0, MULT, ADD)
            uscale(1)
            nc.vector.tensor_mul(cus[3][:], cus[1][:], m3up[:])
            nc.vector.tensor_mul(cuc[3][:], cuc[1][:], m3um[:])
            uscale(3)
            sqv = feat.tile([128, 2, N], f16, tag="sqv")
            m3vp = feat.tile([128, 2, N], f16, tag="m3vp")
            m3vm = feat.tile([128, 2, N], f16, tag="m3vm")
            t2v = feat.tile([128, 2, N], f16, tag="t2v")
            nc.vector.tensor_mul(sqv[:], cvc[1][:], cvc[1][:])
            nc.vector.tensor_scalar(m3vm[:], sqv[:], 4.0, -3.0, MULT, ADD)
            nc.vector.tensor_scalar(m3vp[:], sqv[:], 4.0, -1.0, MULT, ADD)
            nc.vector.tensor_scalar(t2v[:], sqv[:], 4.0, -2.0, MULT, ADD)
            nc.vector.tensor_mul(cvc[3][:], cvc[1][:], m3vm[:])
            nc.vector.tensor_mul(cvs[3][:], cvs[1][:], m3vp[:])
            tu = feat.tile([128, 2, FU], f16, tag="tu")
            nc.vector.tensor_mul(tu[:, 0, :], cus[3][:], t2u[:])
            nc.vector.tensor_mul(tu[:, 1, :], cuc[3][:], t2u[:])
            nc.vector.tensor_sub(cus[5][:], tu[:, 0, :], cus[1][:])
            nc.vector.tensor_sub(cuc[5][:], tu[:, 1, :], cuc[1][:])
            uscale(5)

            # ---- PE score program ----
            # fillers into sc[0] keep HAM warm across the seed window;
            # the j1 start=True MM clears the bank before accumulating
            for _ in range(2):
                nc.tensor.matmul(sc[0][:], ones[:, :128], ones[:, :NH],
                                 skip_group_check=True)
            score_mms(1, first=True)
            score_mms(3)

            # j5 v-chain emitted per key-half, interleaved with the j5
            # score MMs + sigmoid + out MMs of the matching mb pair
            tv = feat.tile([128, 2, 2, N], f16, tag="tv")
            attT = [consts.tile([128, NH], f16, tag=f"attT{mb}", name=f"attT{mb}")
                    for mb in range(4)]
            t2v_b = t2v[:, None, :, :].broadcast_to((128, 2, 2, N))
            cv3l = {0: cvs[3], 1: cvc[3]}
            cv1l = {0: cvs[1], 1: cvc[1]}
            cv5l = {0: cvs[5], 1: cvc[5]}
            for h in range(2):
                sl = slice(h * 256, (h + 1) * 256)
                for ln in range(2):
                    nc.vector.tensor_mul(
                        tv[:, ln, :, sl], cv3l[ln][:, :, sl],
                        t2v_b[:, ln, :, sl])
                    nc.vector.tensor_sub(
                        cv5l[ln][:, :, sl], tv[:, ln, :, sl],
                        cv1l[ln][:, :, sl])
                for mb in (2 * h, 2 * h + 1):
                    for fn in range(2):
                        for c in range(2):
                            lhs = cv5l[1 - fn]
                            rhs = uss[5] if fn == 0 else usc[5]
                            nc.tensor.matmul(
                                sc[mb][:],
                                lhs[:, c, mb * 128:(mb + 1) * 128],
                                rhs[:, c * NH:(c + 1) * NH],
                                start=False,
                                stop=(fn == 1 and c == 1),
                                skip_group_check=True,
                            )
                    nc.scalar.activation(
                        attT[mb][:], sc[mb][:], AF.Tanh, scale=0.5,
                        bias=sgb_sb[:, 0:1]
                    )
                    for nb in range(2):
                        nc.tensor.matmul(
                            fos[nb][:],
                            attT[mb][:, nb * 128:(nb + 1) * 128],
                            xkT_sb[:, mb, :],
                            start=False,
                            stop=(mb == 3),
                            skip_group_check=True,
                        )

            out_sb = opool.tile([128, 2, D], f16, tag="out")
            for nb in range(2):
                nc.scalar.copy(out_sb[:, nb, :], fos[nb][:])
                nc.sync.dma_start(
                    out.ap()[nb * 128:(nb + 1) * 128, :], out_sb[:, nb, :]
                )

    nc.compile()
    return nc


def _prep_inputs_v7(x, Wg1, Wg2, bg, Wa_w, Wa_b, ba, bg_zero):
    """Host-side packing/slicing only (no reference math)."""
    x = np.asarray(x, np.float32)
    w1s = (FS * np.asarray(Wg1, np.float32).T).astype(np.float16)
    w2s = (FS * np.asarray(Wg2, np.float32).T).astype(np.float16)
    wac = np.asarray(Wa_w, np.float32).reshape(2, 128).T
    NBC = 3 if bg_zero else 7
    biasc = np.empty((128, NBC), np.float16)
    biasc[:, 0:2] = wac.astype(np.float16)
    biasc[:, 2] = np.float16(0.5 * (float(np.asarray(Wa_b).ravel()[0])
                                    + float(np.asarray(ba).ravel()[0])))
    if not bg_zero:
        bgv = FS * np.asarray(bg, np.float32)
        biasc[:, 3:5] = bgv.reshape(2, 128).T.astype(np.float16)
        biasc[:, 5:7] = (bgv.reshape(2, 128).T
                         + np.float32(np.pi / 2)).astype(np.float16)
    in_maps = []
    for c in range(NCORES):
        b, half = c // 2, c % 2
        xb = x[b]
        # per-core key permutation: own-half keys first, so xq is a
        # fixed-offset view of xk in every core's (identical) program
        xp = np.concatenate(
            [xb[:, half * NH:(half + 1) * NH],
             xb[:, (1 - half) * NH:(2 - half) * NH]], axis=1)
        vin = np.ascontiguousarray(
            np.concatenate([w1s, xp.astype(np.float16), w2s], axis=1))
        xkTP = ((0.5 * xp.T).astype(np.float16).reshape(4, 128, D)
                .transpose(1, 0, 2).reshape(128, 4 * D))
        xkb = np.ascontiguousarray(np.concatenate([xkTP, biasc], axis=1))
        in_maps.append({"vin": vin, "xkb": xkb})
    return in_maps


def _run(inputs, trace=False):
    from concourse.bass_utils import run_bass_kernel_spmd

    bg_zero = bool(np.all(np.asarray(inputs["bg"]) == 0))
    key = ("nc7b", bg_zero)
    if key not in _cache:
        _cache[key] = _build_nc_v7(bg_zero=bg_zero)
    nc = _cache[key]
    in_maps = _prep_inputs_v7(**inputs, bg_zero=bg_zero)
    res = run_bass_kernel_spmd(
        nc, in_maps, core_ids=list(range(NCORES)), trace=trace
    )
    out = np.empty((B, N, D), np.float32)
    for c in range(NCORES):
        b, half = c // 2, c % 2
        out[b, half * NH:(half + 1) * NH] = \
            res.results[c]["out"].astype(np.float32)
    return out, res


def kernel(**inputs):
    out, _ = _run(inputs, trace=False)
    return out


# revision 9
# speedup vs baseline: 1.0303x; 1.0303x over previous
"""Additive-attention kernel for Trainium2 (8 NeuronCores, SPMD).

Problem (per batch b of B=4):
    xt      = x[b].T                                  # (N=512, D=96)
    g1      = xt @ Wg1.T                              # (512, 256)
    g2      = xt @ Wg2.T                              # (512, 256)
    score   = sum_a Wa[a] * tanh(g1[n,a] + g2[m,a] + bg[a])    # (512, 512)
    att     = sigmoid(score + Wa_b + ba)
    out[b]  = att @ xt                                # (512, 96)

Sharding: core c handles batch b = c//2 and query-rows n in
[(c%2)*256, (c%2)*256+256).  Keys are PERMUTED per core (own-half keys
first) so the query slice xq is a fixed-offset view of the key tile;
the sum over keys is permutation-invariant.

Algorithm (v7): tanh(u+v) ~= sum_{j in 1,3,5} BJ_j * sin(j*S*(u+v)),
S = pi/8.4 (weighted LSQ fit of tanh on |t|<=9, w=N(0,1.4^2)+0.05).
Each harmonic separates, so the N x N score becomes matmuls over a
contraction of (a, j, sin|cos) = 1536.  ACT's sin spline only covers
|x| < 4 (profile exponent buckets), so ONLY the j=1 seeds may use ACT
Sin; j=3,5 come from the f16 DVE recurrence
    f3 = f1*(2cos2t +- 1),  f5 = f3*2cos2t - f1.

Scheduling notes (hard-won):
- Engines only start after the preamble barrier (~7.0us); a HWDGE
  dma_start costs ~0.6-0.7us of descriptor-gen ON the issuing engine
  and the first bytes land ~0.6us later.  vin is ONE transfer on the
  sync ring; xkTP(+f16 bias columns) goes on the scalar HWDGE ring
  right after the silu dummy, so it never blocks vin.
- A single silu-dummy activation forces ONE load of silu_and_others
  (the only ACT table set with sin AND tanh).
- Tile tracks dependencies at TILE granularity, so sin/cos seed lanes
  live in separate tiles (false WAR between ACT lane writes and DVE
  readers otherwise serializes the seed chain).
- PE warms up (HAM 4/8 -> 8/8 needs ~3.4us of sustained activity) on
  ones-matmuls into the thv PSUM region before the real theta MMs.
- sigmoid rewritten 0.5+0.5*tanh(0.5*s): the 0.5 offset becomes a
  ones-colsum matmul and both 0.5 factors are pre-folded into xkTP on
  the host, so out = fos PSUM directly (ACT copies PSUM->SBUF f16).
"""

import numpy as np

B, D, N, A = 4, 96, 512, 256
NH = N // 2          # query rows per core
NCORES = 8

JS = (1, 3, 5)
FL = 8.4
FS = float(np.pi / FL)
BJ = {1: 1.206938, 3: 0.263773, 5: 0.089706}

_cache = {}


def _build_nc_v7(bg_zero=False):
    import concourse.bacc as bacc
    import concourse.mybir as mybir
    from concourse import tile

    f32 = mybir.dt.float32
    f16 = mybir.dt.float16
    AF = mybir.ActivationFunctionType
    MULT = mybir.AluOpType.mult
    ADD = mybir.AluOpType.add

    nc = bacc.Bacc("TRN2", target_bir_lowering=False)

    # xkb = xkTP [128, 4*D] ++ f16 bias columns:
    #   wav(2) sgb(1) [+ bg: b1s(2) b1c(2)]
    NBC = 3 if bg_zero else 7
    vin = nc.dram_tensor("vin", [D, 2 * A + N], f16, kind="ExternalInput")
    xkb = nc.dram_tensor("xkb", [128, 4 * D + NBC], f16,
                         kind="ExternalInput")
    out = nc.dram_tensor("out", [NH, D], f16, kind="ExternalOutput")

    FU = NH * 2          # 512: u-side feature width (2 a-chunks)

    with tile.TileContext(nc) as tc:
        with (
            tc.tile_pool(name="consts", bufs=1) as consts,
            tc.tile_pool(name="feat", bufs=1) as feat,
            tc.tile_pool(name="gps", bufs=1, space="PSUM") as gps,
            tc.tile_pool(name="scps", bufs=1, space="PSUM") as scps,
            tc.tile_pool(name="opool", bufs=1) as opool,
        ):
            vin_sb = consts.tile([D, 2 * A + N], f16, tag="vin")
            xkb_sb = consts.tile([128, 4 * D + NBC], f16, tag="xkb")
            # vin layout: [w1 | xk | w2]; xq = first NH key columns
            w1_sb = vin_sb[:, :A]
            xk_sb = vin_sb[:, A:A + N]
            xq_sb = xk_sb[:, :NH]
            w2_sb = vin_sb[:, A + N:]
            xkT_sb = xkb_sb[:, :4 * D].rearrange("p (mb d) -> p mb d", d=D)
            # bias columns ride as f16; widen to f32 once on-device
            # (tensor_scalar scalar1 and ACT bias APs must be f32)
            biasf = consts.tile([128, NBC], f32, tag="biasf")
            wav_sb = biasf[:, 0:2]
            sgb_sb = biasf[:, 2:3]
            if not bg_zero:
                b1s_sb = biasf[:, 3:5]
                b1c_sb = biasf[:, 5:7]

            # gpsimd owns the const memsets (it is otherwise idle and
            # starts right after the barrier, freeing DVE's queue)
            ones = consts.tile([128, 512], f16, tag="ones")
            nc.gpsimd.memset(ones[:], 1.0)
            hpi = consts.tile([128, 1], f32, tag="hpi")
            nc.gpsimd.memset(hpi[:], float(np.pi / 2))
            dsil = consts.tile([128, 1], f32, tag="dsil")
            nc.gpsimd.memset(dsil[:], 0.0)

            # silu-dummy: forces ONE load of silu_and_others, then the
            # xkb DMA rides the scalar HWDGE ring (vin owns sync's)
            nc.scalar.activation(dsil[:], dsil[:], AF.Silu)
            nc.sync.dma_start(vin_sb[:], vin.ap())
            nc.sync.dma_start(xkb_sb[:], xkb.ap())

            thu = gps.tile([128, FU], f32, tag="thu", name="thu")
            thv = [gps.tile([128, N], f32, tag=f"thv{c}", name=f"thv{c}")
                   for c in range(2)]
            # PE warmup into thv0's region (overwritten by the real MM)
            for _ in range(3):
                nc.tensor.matmul(thv[0][:], ones[:, :128], ones[:])
            for c in range(2):
                nc.tensor.matmul(thu[:, c * NH:(c + 1) * NH],
                                 w1_sb[:, c * 128:(c + 1) * 128], xq_sb[:])
            for c in range(2):
                nc.tensor.matmul(thv[c][:],
                                 w2_sb[:, c * 128:(c + 1) * 128], xk_sb[:])

            # constant half-sum term: fos[nb] starts as sum_m xkTP[m, :]
            fos = [gps.tile([128, D], f32, tag="fo", name=f"fo{nb}")
                   for nb in range(2)]
            for mb in range(4):
                for nb in range(2):
                    nc.tensor.matmul(
                        fos[nb][:], ones[:, :128], xkT_sb[:, mb, :],
                        start=(mb == 0), stop=False, skip_group_check=True,
                    )

            # per-lane feature tiles (separate tiles per sin|cos lane so
            # ACT lane writes never false-serialize against DVE readers)
            # u side: [128, FU]; v side: [128, 2(chunk), N]
            cus = {j: feat.tile([128, FU], f16, tag=f"cus{j}", name=f"cus{j}") for j in JS}
            cuc = {j: feat.tile([128, FU], f16, tag=f"cuc{j}", name=f"cuc{j}") for j in JS}
            cvs = {j: feat.tile([128, 2, N], f16, tag=f"cvs{j}", name=f"cvs{j}") for j in JS}
            cvc = {j: feat.tile([128, 2, N], f16, tag=f"cvc{j}", name=f"cvc{j}") for j in JS}
            uss = {j: feat.tile([128, FU], f16, tag=f"uss{j}", name=f"uss{j}") for j in JS}
            usc = {j: feat.tile([128, FU], f16, tag=f"usc{j}", name=f"usc{j}") for j in JS}

            def useed(lane, tile_):
                if bg_zero:
                    bias = hpi[:] if lane else 0.0
                    nc.scalar.activation(tile_[:], thu[:], AF.Sin, bias=bias)
                else:
                    bl = b1c_sb if lane else b1s_sb
                    for c in range(2):
                        nc.scalar.activation(
                            tile_[:, c * NH:(c + 1) * NH],
                            thu[:, c * NH:(c + 1) * NH], AF.Sin,
                            bias=bl[:, c:c + 1])

            def vseed(lane, tile_, c):
                if bg_zero:
                    bias = hpi[:] if lane else 0.0
                else:
                    bias = (b1c_sb if lane else b1s_sb)[:, c:c + 1]
                nc.scalar.activation(tile_[:, c, :], thv[c][:], AF.Sin,
                                     bias=bias)

            useed(1, cuc[1])
            useed(0, cus[1])
            vseed(1, cvc[1], 0)
            vseed(1, cvc[1], 1)
            vseed(0, cvs[1], 0)
            vseed(0, cvs[1], 1)

            def uscale(j):
                for c in range(2):
                    for st, ct in ((uss, cus), (usc, cuc)):
                        nc.vector.tensor_scalar(
                            st[j][:, c * NH:(c + 1) * NH],
                            ct[j][:, c * NH:(c + 1) * NH],
                            wav_sb[:, c:c + 1], float(BJ[j]), MULT, MULT)

            sc = [scps.tile([128, NH], f32, tag=f"sc{mb}", name=f"sc{mb}")
                  for mb in range(4)]

            def score_mms(j, first=False):
                lhs = {0: cvc[j], 1: cvs[j]}  # fn=0 pairs us-sin x cv-cos
                rhs = {0: uss[j], 1: usc[j]}
                for fn in range(2):
                    for c in range(2):
                        for mb in range(4):
                            nc.tensor.matmul(
                                sc[mb][:],
                                lhs[fn][:, c, mb * 128:(mb + 1) * 128],
                                rhs[fn][:, c * NH:(c + 1) * NH],
                                start=(first and fn == 0 and c == 0),
                                stop=False,
                                skip_group_check=True,
                            )

            # ---- DVE program (emission order = engine order) ----
            squ = feat.tile([128, FU], f16, tag="squ")
            m3up = feat.tile([128, FU], f16, tag="m3up")
            m3um = feat.tile([128, FU], f16, tag="m3um")
            t2u = feat.tile([128, FU], f16, tag="t2u")
            nc.vector.tensor_copy(biasf[:], xkb_sb[:, 4 * D:])
            nc.vector.tensor_mul(squ[:], cuc[1][:], cuc[1][:])
            nc.vector.tensor_scalar(m3up[:], squ[:], 4.0, -1.0, MULT, ADD)
            nc.vector.tensor_scalar(m3um[:], squ[:], 4.0, -3.0, MULT, ADD)
            nc.vector.tensor_scalar(t2u[:], squ[:], 4.0, -2.0, MULT, ADD)
            uscale(1)
            nc.vector.tensor_mul(cus[3][:], cus[1][:], m3up[:])
            nc.vector.tensor_mul(cuc[3][:], cuc[1][:], m3um[:])
            uscale(3)
            sqv = feat.tile([128, 2, N], f16, tag="sqv")
            m3vp = feat.tile([128, 2, N], f16, tag="m3vp")
            m3vm = feat.tile([128, 2, N], f16, tag="m3vm")
            t2v = feat.tile([128, 2, N], f16, tag="t2v")
            nc.vector.tensor_mul(sqv[:], cvc[1][:], cvc[1][:])
            nc.vector.tensor_scalar(m3vm[:], sqv[:], 4.0, -3.0, MULT, ADD)
            nc.vector.tensor_scalar(m3vp[:], sqv[:], 4.0, -1.0, MULT, ADD)
            nc.vector.tensor_scalar(t2v[:], sqv[:], 4.0, -2.0, MULT, ADD)
            nc.vector.tensor_mul(cvc[3][:], cvc[1][:], m3vm[:])
            nc.vector.tensor_mul(cvs[3][:], cvs[1][:], m3vp[:])
            tu = feat.tile([128, 2, FU], f16, tag="tu")
            nc.vector.tensor_mul(tu[:, 0, :], cus[3][:], t2u[:])
            nc.vector.tensor_mul(tu[:, 1, :], cuc[3][:], t2u[:])
            nc.vector.tensor_sub(cus[5][:], tu[:, 0, :], cus[1][:])
            nc.vector.tensor_sub(cuc[5][:], tu[:, 1, :], cuc[1][:])
            uscale(5)

            # ---- PE score program ----
            # fillers into sc[0] keep HAM warm across the seed window;
            # the j1 start=True MM clears the bank before accumulating
            for _ in range(2):
                nc.tensor.matmul(sc[0][:], ones[:, :128], ones[:, :NH],
                                 skip_group_check=True)
            score_mms(1, first=True)
            score_mms(3)

            # j5 v-chain emitted per key-half, interleaved with the j5
            # score MMs + sigmoid + out MMs of the matching mb pair
            tv = feat.tile([128, 2, 2, N], f16, tag="tv")
            attT = [consts.tile([128, NH], f16, tag=f"attT{mb}", name=f"attT{mb}")
                    for mb in range(4)]
            t2v_b = t2v[:, None, :, :].broadcast_to((128, 2, 2, N))
            cv3l = {0: cvs[3], 1: cvc[3]}
            cv1l = {0: cvs[1], 1: cvc[1]}
            cv5l = {0: cvs[5], 1: cvc[5]}
            for h in range(2):
                sl = slice(h * 256, (h + 1) * 256)
                for ln in range(2):
                    nc.vector.tensor_mul(
                        tv[:, ln, :, sl], cv3l[ln][:, :, sl],
                        t2v_b[:, ln, :, sl])
                    nc.vector.tensor_sub(
                        cv5l[ln][:, :, sl], tv[:, ln, :, sl],
                        cv1l[ln][:, :, sl])
                for mb in (2 * h, 2 * h + 1):
                    for fn in range(2):
                        for c in range(2):
                            lhs = cv5l[1 - fn]
                            rhs = uss[5] if fn == 0 else usc[5]
                            nc.tensor.matmul(
                                sc[mb][:],
                                lhs[:, c, mb * 128:(mb + 1) * 128],
                                rhs[:, c * NH:(c + 1) * NH],
                                start=False,
                                stop=(fn == 1 and c == 1),
                                skip_group_check=True,
                            )
                    nc.scalar.activation(
                        attT[mb][:], sc[mb][:], AF.Tanh, scale=0.5,
                        bias=sgb_sb[:, 0:1]
                    )
                    for nb in range(2):
                        nc.tensor.matmul(
                            fos[nb][:],
                            attT[mb][:, nb * 128:(nb + 1) * 128],
                            xkT_sb[:, mb, :],
                            start=False,
                            stop=(mb == 3),
                            skip_group_check=True,
                        )

            out_sb = opool.tile([128, 2, D], f16, tag="out")
            for nb in range(2):
                nc.scalar.copy(out_sb[:, nb, :], fos[nb][:])
                nc.sync.dma_start(
                    out.ap()[nb * 128:(nb + 1) * 128, :], out_sb[:, nb, :]
                )

    nc.compile()
    return nc


def _prep_inputs_v7(x, Wg1, Wg2, bg, Wa_w, Wa_b, ba, bg_zero):
    """Host-side packing/slicing only (no reference math)."""
    x = np.asarray(x, np.float32)
    w1s = (FS * np.asarray(Wg1, np.float32).T).astype(np.float16)
    w2s = (FS * np.asarray(Wg2, np.float32).T).astype(np.float16)
    wac = np.asarray(Wa_w, np.float32).reshape(2, 128).T
    NBC = 3 if bg_zero else 7
    biasc = np.empty((128, NBC), np.float16)
    biasc[:, 0:2] = wac.astype(np.float16)
    biasc[:, 2] = np.float16(0.5 * (float(np.asarray(Wa_b).ravel()[0])
                                    + float(np.asarray(ba).ravel()[0])))
    if not bg_zero:
        bgv = FS * np.asarray(bg, np.float32)
        biasc[:, 3:5] = bgv.reshape(2, 128).T.astype(np.float16)
        biasc[:, 5:7] = (bgv.reshape(2, 128).T
                         + np.float32(np.pi / 2)).astype(np.float16)
    in_maps = []
    for c in range(NCORES):
        b, half = c // 2, c % 2
        xb = x[b]
        # per-core key permutation: own-half keys first, so xq is a
        # fixed-offset view of xk in every core's (identical) program
        xp = np.concatenate(
            [xb[:, half * NH:(half + 1) * NH],
             xb[:, (1 - half) * NH:(2 - half) * NH]], axis=1)
        vin = np.ascontiguousarray(
            np.concatenate([w1s, xp.astype(np.float16), w2s], axis=1))
        xkTP = ((0.5 * xp.T).astype(np.float16).reshape(4, 128, D)
                .transpose(1, 0, 2).reshape(128, 4 * D))
        xkb = np.ascontiguousarray(np.concatenate([xkTP, biasc], axis=1))
        in_maps.append({"vin": vin, "xkb": xkb})
    return in_maps


def _run(inputs, trace=False):
    from concourse.bass_utils import run_bass_kernel_spmd

    bg_zero = bool(np.all(np.asarray(inputs["bg"]) == 0))
    key = ("nc7b", bg_zero)
    if key not in _cache:
        _cache[key] = _build_nc_v7(bg_zero=bg_zero)
    nc = _cache[key]
    in_maps = _prep_inputs_v7(**inputs, bg_zero=bg_zero)
    res = run_bass_kernel_spmd(
        nc, in_maps, core_ids=list(range(NCORES)), trace=trace
    )
    out = np.empty((B, N, D), np.float32)
    for c in range(NCORES):
        b, half = c // 2, c % 2
        out[b, half * NH:(half + 1) * NH] = \
            res.results[c]["out"].astype(np.float32)
    return out, res


def kernel(**inputs):
    out, _ = _run(inputs, trace=False)
    return out


# revision 11
# speedup vs baseline: 1.0601x; 1.0289x over previous
"""Additive-attention kernel for Trainium2 (8 NeuronCores, SPMD).

Problem (per batch b of B=4):
    xt      = x[b].T                                  # (N=512, D=96)
    g1      = xt @ Wg1.T                              # (512, 256)
    g2      = xt @ Wg2.T                              # (512, 256)
    score   = sum_a Wa[a] * tanh(g1[n,a] + g2[m,a] + bg[a])    # (512, 512)
    att     = sigmoid(score + Wa_b + ba)
    out[b]  = att @ xt                                # (512, 96)

Sharding: core c handles batch b = c//2 and query-rows n in
[(c%2)*256, (c%2)*256+256).  Keys are PERMUTED per core (own-half keys
first) so the query slice xq is a fixed-offset view of the key tile;
the sum over keys is permutation-invariant.

Algorithm (v7): tanh(u+v) ~= sum_{j in 1,3,5} BJ_j * sin(j*S*(u+v)),
S = pi/8.4 (weighted LSQ fit of tanh on |t|<=9, w=N(0,1.4^2)+0.05).
Each harmonic separates, so the N x N score becomes matmuls over a
contraction of (a, j, sin|cos) = 1536.  ACT's sin spline only covers
|x| < 4 (profile exponent buckets), so ONLY the j=1 seeds may use ACT
Sin; j=3,5 come from the f16 DVE recurrence
    f3 = f1*(2cos2t +- 1),  f5 = f3*2cos2t - f1.

Scheduling notes (hard-won):
- Engines only start after the preamble barrier (~7.0us); a HWDGE
  dma_start costs ~0.6-0.7us of descriptor-gen ON the issuing engine
  and the first bytes land ~0.6us later.  vin is ONE transfer on the
  sync ring; xkTP(+f16 bias columns) goes on the scalar HWDGE ring
  right after the silu dummy, so it never blocks vin.
- A single silu-dummy activation forces ONE load of silu_and_others
  (the only ACT table set with sin AND tanh).
- Tile tracks dependencies at TILE granularity, so sin/cos seed lanes
  live in separate tiles (false WAR between ACT lane writes and DVE
  readers otherwise serializes the seed chain).
- PE warms up (HAM 4/8 -> 8/8 needs ~3.4us of sustained activity) on
  ones-matmuls into the thv PSUM region before the real theta MMs.
- sigmoid rewritten 0.5+0.5*tanh(0.5*s): the 0.5 offset becomes a
  ones-colsum matmul and both 0.5 factors are pre-folded into xkTP on
  the host, so out = fos PSUM directly (ACT copies PSUM->SBUF f16).
"""

import numpy as np

B, D, N, A = 4, 96, 512, 256
NH = N // 2          # query rows per core
NCORES = 8

JS = (1, 3, 5)
FL = 8.4
FS = float(np.pi / FL)
BJ = {1: 1.206938, 3: 0.263773, 5: 0.089706}

_cache = {}


def _build_nc_v7(bg_zero=False):
    import concourse.bacc as bacc
    import concourse.mybir as mybir
    from concourse import tile

    f32 = mybir.dt.float32
    f16 = mybir.dt.float16
    AF = mybir.ActivationFunctionType
    MULT = mybir.AluOpType.mult
    ADD = mybir.AluOpType.add

    nc = bacc.Bacc("TRN2", target_bir_lowering=False)

    # xkb = xkTP [128, 4*D] ++ f16 bias columns:
    #   wav(2) sgb(1) [+ bg: b1s(2) b1c(2)]
    NBC = 3 if bg_zero else 7
    vin = nc.dram_tensor("vin", [D, 2 * A + N], f16, kind="ExternalInput")
    xkb = nc.dram_tensor("xkb", [128, 4 * D + NBC], f16,
                         kind="ExternalInput")
    out = nc.dram_tensor("out", [NH, D], f16, kind="ExternalOutput")

    FU = NH * 2          # 512: u-side feature width (2 a-chunks)

    with tile.TileContext(nc) as tc:
        with (
            tc.tile_pool(name="consts", bufs=1) as consts,
            tc.tile_pool(name="feat", bufs=1) as feat,
            tc.tile_pool(name="gps", bufs=1, space="PSUM") as gps,
            tc.tile_pool(name="scps", bufs=1, space="PSUM") as scps,
            tc.tile_pool(name="opool", bufs=1) as opool,
        ):
            vin_sb = consts.tile([D, 2 * A + N], f16, tag="vin")
            xkb_sb = consts.tile([128, 4 * D + NBC], f16, tag="xkb")
            # vin layout: [w1 | xk | w2]; xq = first NH key columns
            w1_sb = vin_sb[:, :A]
            xk_sb = vin_sb[:, A:A + N]
            xq_sb = xk_sb[:, :NH]
            w2_sb = vin_sb[:, A + N:]
            xkT_sb = xkb_sb[:, :4 * D].rearrange("p (mb d) -> p mb d", d=D)
            # bias columns ride as f16; widen to f32 once on-device
            # (tensor_scalar scalar1 and ACT bias APs must be f32)
            biasf = consts.tile([128, NBC], f32, tag="biasf")
            wav_sb = biasf[:, 0:2]
            sgb_sb = biasf[:, 2:3]
            if not bg_zero:
                b1s_sb = biasf[:, 3:5]
                b1c_sb = biasf[:, 5:7]

            # gpsimd owns the const memsets (it is otherwise idle and
            # starts right after the barrier, freeing DVE's queue)
            ones = consts.tile([128, 512], f16, tag="ones")
            nc.gpsimd.memset(ones[:], 1.0)
            hpi = consts.tile([128, 1], f32, tag="hpi")
            nc.gpsimd.memset(hpi[:], float(np.pi / 2))
            dsil = consts.tile([128, 1], f32, tag="dsil")
            nc.gpsimd.memset(dsil[:], 0.0)

            # silu-dummy: forces ONE load of silu_and_others, then the
            # xkb DMA rides the scalar HWDGE ring (vin owns sync's)
            nc.scalar.activation(dsil[:], dsil[:], AF.Silu)
            nc.sync.dma_start(vin_sb[:], vin.ap())
            nc.sync.dma_start(xkb_sb[:], xkb.ap())

            thu = gps.tile([128, FU], f32, tag="thu", name="thu")
            thv = [gps.tile([128, N], f32, tag=f"thv{c}", name=f"thv{c}")
                   for c in range(2)]
            # PE warmup into thv0's region (overwritten by the real MM)
            for _ in range(2):
                nc.tensor.matmul(thv[0][:], ones[:, :128], ones[:])
            for c in range(2):
                nc.tensor.matmul(thu[:, c * NH:(c + 1) * NH],
                                 w1_sb[:, c * 128:(c + 1) * 128], xq_sb[:])
            for c in range(2):
                nc.tensor.matmul(thv[c][:],
                                 w2_sb[:, c * 128:(c + 1) * 128], xk_sb[:])

            # constant half-sum term: fos[nb] starts as sum_m xkTP[m, :]
            fos = [gps.tile([128, D], f32, tag="fo", name=f"fo{nb}")
                   for nb in range(2)]
            for mb in range(4):
                for nb in range(2):
                    nc.tensor.matmul(
                        fos[nb][:], ones[:, :128], xkT_sb[:, mb, :],
                        start=(mb == 0), stop=False, skip_group_check=True,
                    )

            # per-lane feature tiles (separate tiles per sin|cos lane so
            # ACT lane writes never false-serialize against DVE readers)
            # u side: [128, FU]; v side: [128, 2(chunk), N]
            # u-side tiles carry both lanes [128, 2(sin|cos), FU] (all
            # DVE-written, so no cross-engine false deps); v-side seed
            # and j3 tiles are split per lane (ACT lane writes / early
            # cos-lane consumers), cv5 is one both-lane tile.
            cu = {j: feat.tile([128, 2, FU], f16, tag=f"cu{j}", name=f"cu{j}")
                  for j in JS}
            us = {j: feat.tile([128, 2, FU], f16, tag=f"us{j}", name=f"us{j}")
                  for j in JS}
            cvs = {j: feat.tile([128, 2, N], f16, tag=f"cvs{j}", name=f"cvs{j}") for j in (1, 3)}
            cvc = {j: feat.tile([128, 2, N], f16, tag=f"cvc{j}", name=f"cvc{j}") for j in (1, 3)}
            cv5 = feat.tile([128, 2, 2, N], f16, tag="cv5", name="cv5")

            def useed(lane):
                if bg_zero:
                    bias = hpi[:] if lane else 0.0
                    nc.scalar.activation(cu[1][:, lane, :], thu[:], AF.Sin,
                                         bias=bias)
                else:
                    bl = b1c_sb if lane else b1s_sb
                    for c in range(2):
                        nc.scalar.activation(
                            cu[1][:, lane, c * NH:(c + 1) * NH],
                            thu[:, c * NH:(c + 1) * NH], AF.Sin,
                            bias=bl[:, c:c + 1])

            def vseed(lane, tile_, c):
                if bg_zero:
                    bias = hpi[:] if lane else 0.0
                else:
                    bias = (b1c_sb if lane else b1s_sb)[:, c:c + 1]
                nc.scalar.activation(tile_[:, c, :], thv[c][:], AF.Sin,
                                     bias=bias)

            useed(1)
            useed(0)
            vseed(1, cvc[1], 0)
            vseed(1, cvc[1], 1)
            vseed(0, cvs[1], 0)
            vseed(0, cvs[1], 1)

            def uscale(j):
                for c in range(2):
                    nc.vector.tensor_scalar(
                        us[j][:, :, c * NH:(c + 1) * NH],
                        cu[j][:, :, c * NH:(c + 1) * NH],
                        wav_sb[:, c:c + 1], float(BJ[j]), MULT, MULT)

            sc = [scps.tile([128, NH], f32, tag=f"sc{mb}", name=f"sc{mb}")
                  for mb in range(4)]

            def score_mms(j, first=False):
                lhs = {0: cvc[j], 1: cvs[j]}  # fn=0 pairs us-sin x cv-cos
                for fn in range(2):
                    for c in range(2):
                        for mb in range(4):
                            nc.tensor.matmul(
                                sc[mb][:],
                                lhs[fn][:, c, mb * 128:(mb + 1) * 128],
                                us[j][:, fn, c * NH:(c + 1) * NH],
                                start=(first and fn == 0 and c == 0),
                                stop=False,
                                skip_group_check=True,
                            )

            # ---- DVE program (emission order = engine order) ----
            squ = feat.tile([128, FU], f16, tag="squ")
            t2u = feat.tile([128, FU], f16, tag="t2u")
            m3u = feat.tile([128, 2, FU], f16, tag="m3u")
            nc.vector.tensor_copy(biasf[:], xkb_sb[:, 4 * D:])
            nc.vector.tensor_mul(squ[:], cu[1][:, 1, :], cu[1][:, 1, :])
            nc.vector.tensor_scalar(t2u[:], squ[:], 4.0, -2.0, MULT, ADD)
            uscale(1)
            nc.vector.tensor_scalar(m3u[:, 0, :], squ[:], 4.0, -1.0,
                                    MULT, ADD)
            nc.vector.tensor_scalar(m3u[:, 1, :], squ[:], 4.0, -3.0,
                                    MULT, ADD)
            nc.vector.tensor_mul(cu[3][:], cu[1][:], m3u[:])
            uscale(3)
            sqv = feat.tile([128, 2, N], f16, tag="sqv")
            m3vp = feat.tile([128, 2, N], f16, tag="m3vp")
            m3vm = feat.tile([128, 2, N], f16, tag="m3vm")
            t2v = feat.tile([128, 2, N], f16, tag="t2v")
            nc.vector.tensor_mul(sqv[:], cvc[1][:], cvc[1][:])
            nc.vector.tensor_scalar(t2v[:], sqv[:], 4.0, -2.0, MULT, ADD)
            nc.vector.tensor_scalar(m3vm[:], sqv[:], 4.0, -3.0, MULT, ADD)
            nc.vector.tensor_mul(cvc[3][:], cvc[1][:], m3vm[:])
            nc.vector.tensor_scalar(m3vp[:], sqv[:], 4.0, -1.0, MULT, ADD)
            nc.vector.tensor_mul(cvs[3][:], cvs[1][:], m3vp[:])
            # u5 recurrence (merged lanes): cu5 = cu3*2cos2t - cu1
            t2u_b = t2u[:, None, :].broadcast_to((128, 2, FU))
            tu = feat.tile([128, 2, FU], f16, tag="tu")
            nc.vector.tensor_mul(tu[:], cu[3][:], t2u_b)
            nc.vector.tensor_sub(cu[5][:], tu[:], cu[1][:])
            uscale(5)
            # ---- PE score program ----
            # fillers into sc[0] keep HAM warm across the seed window;
            # the j1 start=True MM clears the bank before accumulating
            for _ in range(4):
                nc.tensor.matmul(sc[0][:], ones[:, :128], ones[:, :NH],
                                 skip_group_check=True)
            score_mms(1, first=True)
            score_mms(3)

            # j5 v-chain emitted per key-half, interleaved with the j5
            # score MMs + sigmoid + out MMs of the matching mb pair
            tv = feat.tile([128, 2, 2, N], f16, tag="tv")
            attT = [consts.tile([128, NH], f16, tag=f"attT{mb}", name=f"attT{mb}")
                    for mb in range(4)]
            t2v_b = t2v[:, None, :, :].broadcast_to((128, 2, 2, N))
            cv5l = {1: cv5[:, 1], 0: cv5[:, 0]}
            for h in range(2):
                sl = slice(h * 256, (h + 1) * 256)
                # cos lane first: it is the fn=0 lhsT
                for ln, c3, c1 in ((1, cvc[3], cvc[1]), (0, cvs[3], cvs[1])):
                    nc.vector.tensor_mul(
                        tv[:, ln, :, sl], c3[:, :, sl], t2v_b[:, ln, :, sl])
                    nc.vector.tensor_sub(
                        cv5[:, ln, :, sl], tv[:, ln, :, sl], c1[:, :, sl])
                for mb in (2 * h, 2 * h + 1):
                    for fn in range(2):
                        for c in range(2):
                            nc.tensor.matmul(
                                sc[mb][:],
                                cv5[:, 1 - fn, c, mb * 128:(mb + 1) * 128],
                                us[5][:, fn, c * NH:(c + 1) * NH],
                                start=False,
                                stop=(fn == 1 and c == 1),
                                skip_group_check=True,
                            )
                    nc.scalar.activation(
                        attT[mb][:], sc[mb][:], AF.Tanh, scale=0.5,
                        bias=sgb_sb[:, 0:1]
                    )
                    for nb in range(2):
                        nc.tensor.matmul(
                            fos[nb][:],
                            attT[mb][:, nb * 128:(nb + 1) * 128],
                            xkT_sb[:, mb, :],
                            start=False,
                            stop=(mb == 3),
                            skip_group_check=True,
                        )

            out_sb = opool.tile([128, 2, D], f16, tag="out")
            for nb in range(2):
                nc.scalar.copy(out_sb[:, nb, :], fos[nb][:])
                nc.sync.dma_start(
                    out.ap()[nb * 128:(nb + 1) * 128, :], out_sb[:, nb, :]
                )

    nc.compile()
    return nc


def _prep_inputs_v7(x, Wg1, Wg2, bg, Wa_w, Wa_b, ba, bg_zero):
    """Host-side packing/slicing only (no reference math)."""
    x = np.asarray(x, np.float32)
    w1s = (FS * np.asarray(Wg1, np.float32).T).astype(np.float16)
    w2s = (FS * np.asarray(Wg2, np.float32).T).astype(np.float16)
    wac = np.asarray(Wa_w, np.float32).reshape(2, 128).T
    NBC = 3 if bg_zero else 7
    biasc = np.empty((128, NBC), np.float16)
    biasc[:, 0:2] = wac.astype(np.float16)
    biasc[:, 2] = np.float16(0.5 * (float(np.asarray(Wa_b).ravel()[0])
                                    + float(np.asarray(ba).ravel()[0])))
    if not bg_zero:
        bgv = FS * np.asarray(bg, np.float32)
        biasc[:, 3:5] = bgv.reshape(2, 128).T.astype(np.float16)
        biasc[:, 5:7] = (bgv.reshape(2, 128).T
                         + np.float32(np.pi / 2)).astype(np.float16)
    in_maps = []
    for c in range(NCORES):
        b, half = c // 2, c % 2
        xb = x[b]
        # per-core key permutation: own-half keys first, so xq is a
        # fixed-offset view of xk in every core's (identical) program
        xp = np.concatenate(
            [xb[:, half * NH:(half + 1) * NH],
             xb[:, (1 - half) * NH:(2 - half) * NH]], axis=1)
        vin = np.ascontiguousarray(
            np.concatenate([w1s, xp.astype(np.float16), w2s], axis=1))
        xkTP = ((0.5 * xp.T).astype(np.float16).reshape(4, 128, D)
                .transpose(1, 0, 2).reshape(128, 4 * D))
        xkb = np.ascontiguousarray(np.concatenate([xkTP, biasc], axis=1))
        in_maps.append({"vin": vin, "xkb": xkb})
    return in_maps


def _run(inputs, trace=False):
    from concourse.bass_utils import run_bass_kernel_spmd

    bg_zero = bool(np.all(np.asarray(inputs["bg"]) == 0))
    key = ("nc7b", bg_zero)
    if key not in _cache:
        _cache[key] = _build_nc_v7(bg_zero=bg_zero)
    nc = _cache[key]
    in_maps = _prep_inputs_v7(**inputs, bg_zero=bg_zero)
    res = run_bass_kernel_spmd(
        nc, in_maps, core_ids=list(range(NCORES)), trace=trace
    )
    out = np.empty((B, N, D), np.float32)
    for c in range(NCORES):
        b, half = c // 2, c % 2
        out[b, half * NH:(half + 1) * NH] = \
            res.results[c]["out"].astype(np.float32)
    return out, res


def kernel(**inputs):
    out, _ = _run(inputs, trace=False)
    return out
